# revision 27
# baseline (speedup 1.0000x reference)
"""Trainium2 Bass kernel for a 3-block GPT (B=2,T=2048,E=1024,H=16,V=32000).

Sharding: sequence-parallel over 8 cores (512 tokens each, weights replicated).
Per layer: QKV local, ONE group-local AllGather of merged {K^T, V} in fp8
(per-batch groups [[0-3],[4-7]] so the SPMD program is identical on every
core), attention in scores-transposed layout (softmax denominator via
ones-matmul), proj/FFN with weights stationary, LayerNorm in transposed layout
via fp32r ones-matmul partition reductions. lm_head produces logits^T [V, 512]
per core; host reassembles. Per-core differences (token ids, causal masks)
enter via input data only.
"""

import numpy as np
import ml_dtypes
from contextlib import ExitStack

import concourse.bass as bass
import concourse.mybir as mybir
import concourse.tile as tile
from concourse import bacc
from concourse.masks import make_identity
from concourse import bass_utils

# model dims (hardcoded; harness contract)
B, T, E, H, V = 2, 2048, 1024, 16, 32000
HD, L = 64, 3
NC = 8
S = (B * T) // NC        # 512 tokens per core
CH = NC // B             # 4 chunks (cores) per batch
FF = 4 * E               # 4096
EPS = 1e-5
P = 128
NE = E // P              # 8 e-tiles
NHP = H // 2             # 8 head pairs
NSB = T // P             # 16 key blocks per batch
NVT = V // P             # 250 vocab tiles
NFT = FF // P            # 32 f-tiles
ES = E * S               # elements in one K slab
VROW = H * 65            # 1040: v row layout = 16x(64 values + ones col)
VS = S * VROW            # elements in one V slab
F32 = mybir.dt.float32
F32R = mybir.dt.float32r
BF16 = mybir.dt.bfloat16
FP8 = mybir.dt.float8e4
I32 = mybir.dt.int32
AF = mybir.ActivationFunctionType
OP = mybir.AluOpType

KV_DT = BF16             # dtype of the gathered K/V slabs


def _ln_tiles(nc, tc, pools, src, out, g_t, b_t, ones_f, eps_t):
    """LayerNorm over E (partition axis across the 8 [128,S] tiles of src).

    src/out: lists of 8 SBUF tiles [128, S] (xT layout). Stats via fp32r
    ones-matmul partition reduction broadcast to all 128 partitions."""
    pacc, tp = pools["pacc"], pools["tp"]
    psm = pacc.tile([P, S], F32, tag="ps_a", name="ln_ps", bufs=2)
    pss = pacc.tile([P, S], F32, tag="ps_a", name="ln_ps", bufs=2)
    for e in range(NE):
        xc = tp.tile([P, S], BF16, tag="ln_xc", name="ln_xc", bufs=2)
        nc.gpsimd.tensor_copy(xc[:], src[e][:])
        nc.tensor.matmul(psm[:], lhsT=ones_f[:], rhs=xc[:],
                         start=(e == 0), stop=(e == NE - 1), skip_group_check=True)
        sq = tp.tile([P, S], BF16, tag="ln_sq", name="ln_sq", bufs=2)
        nc.scalar.square(sq[:], src[e][:])
        nc.tensor.matmul(pss[:], lhsT=ones_f[:], rhs=sq[:],
                         start=(e == 0), stop=(e == NE - 1), skip_group_check=True)
    mean = tp.tile([P, S], F32, tag="ln_mean", name="ln_mean")
    nc.scalar.mul(mean[:], psm[:], 1.0 / E)
    msq = tp.tile([P, S], F32, tag="ln_msq", name="ln_msq")
    nc.scalar.square(msq[:], mean[:])
    var = tp.tile([P, S], F32, tag="ln_var", name="ln_var")
    nc.vector.tensor_scalar(var[:], pss[:], 1.0 / E, None, OP.mult)
    nc.vector.tensor_tensor(out=var[:], in0=var[:], in1=msq[:], op=OP.subtract)
    std = tp.tile([P, S], F32, tag="ln_std", name="ln_std")
    nc.scalar.activation(std[:], var[:], AF.Sqrt, bias=eps_t[:])
    rstd = tp.tile([P, S], F32, tag="ln_rstd", name="ln_rstd")
    nc.vector.reciprocal(rstd[:], std[:])
    for e in range(NE):
        t = tp.tile([P, S], F32, tag="ln_t", name="ln_t", bufs=2)
        nc.vector.tensor_tensor(out=t[:], in0=src[e][:], in1=mean[:], op=OP.subtract)
        nc.vector.tensor_tensor(out=t[:], in0=t[:], in1=rstd[:], op=OP.mult)
        nc.vector.tensor_scalar(out[e][:], t[:], g_t[:, e:e + 1],
                                b_t[:, e:e + 1], OP.mult, OP.add)


def build_program():
    nc = bacc.Bacc("TRN2", target_bir_lowering=False, debug=False, num_devices=NC)

    # ---- DRAM I/O ----
    # weights are host-pre-swizzled into the exact SBUF tile layouts so every
    # weight DMA is a single fully-contiguous transfer (elem >= 4KB)
    d_wqr = nc.dram_tensor("wqr", [L, NE // 2, P, 2, NE, P], BF16, kind="ExternalInput")
    d_wkr = nc.dram_tensor("wkr", [L, NE // 2, P, 2, NE, P], BF16, kind="ExternalInput")
    d_wvf = nc.dram_tensor("wvf", [L, 2, P, NE, 512], BF16, kind="ExternalInput")
    d_wor = nc.dram_tensor("wor", [L, NE // 2, 65, 2, 2 * NE, P], BF16, kind="ExternalInput")
    d_w1r = nc.dram_tensor("w1r", [L, NFT // 2, P, 2, NE, P], BF16, kind="ExternalInput")
    d_w2r = nc.dram_tensor("w2r", [L, NE, P, NFT, P], BF16, kind="ExternalInput")
    d_wlm = nc.dram_tensor("wlmr", [NVT // 2, P, 2, NE, P], BF16, kind="ExternalInput")
    d_emb = nc.dram_tensor("emb", [V, E], F32, kind="ExternalInput")
    d_idx = nc.dram_tensor("idx", [S], I32, kind="ExternalInput")
    d_msk = nc.dram_tensor("maskp", [24, P, S // 2], BF16, kind="ExternalInput")
    d_ln1g = nc.dram_tensor("ln1g", [L, E], F32, kind="ExternalInput")
    d_ln1b = nc.dram_tensor("ln1b", [L, E], F32, kind="ExternalInput")
    d_ln2g = nc.dram_tensor("ln2g", [L, E], F32, kind="ExternalInput")
    d_ln2b = nc.dram_tensor("ln2b", [L, E], F32, kind="ExternalInput")
    d_bo = nc.dram_tensor("bo", [L, E], F32, kind="ExternalInput")
    d_b1 = nc.dram_tensor("b1", [L, FF], F32, kind="ExternalInput")
    d_b2 = nc.dram_tensor("b2", [L, E], F32, kind="ExternalInput")
    d_lnfg = nc.dram_tensor("lnfg", [E], F32, kind="ExternalInput")
    d_lnfb = nc.dram_tensor("lnfb", [E], F32, kind="ExternalInput")
    d_blm = nc.dram_tensor("blm", [V], F32, kind="ExternalInput")
    d_out = nc.dram_tensor("logt", [V, S], F32, kind="ExternalOutput")

    groups = [[0, 1, 2, 3], [4, 5, 6, 7]]

    with ExitStack() as ctx:
        tc = ctx.enter_context(tile.TileContext(nc, num_cores=NC))
        const = ctx.enter_context(tc.tile_pool(name="const", bufs=1))
        pp_x = ctx.enter_context(tc.tile_pool(name="xres", bufs=1))
        pp_sum = ctx.enter_context(tc.tile_pool(name="xsum", bufs=1))
        pp_msk = ctx.enter_context(tc.tile_pool(name="masks", bufs=1))
        tp = ctx.enter_context(tc.tile_pool(name="tp", bufs=1))
        wp = ctx.enter_context(tc.tile_pool(name="wstream", bufs=1))
        pacc = ctx.enter_context(tc.tile_pool(name="pacc", bufs=1, space="PSUM"))
        dram = ctx.enter_context(tc.tile_pool(name="ccdram", bufs=2, space="DRAM"))
        pools = {"pacc": pacc, "tp": tp}

        ident = const.tile([P, P], F32, name="ident")
        make_identity(nc, ident[:])
        ones_f = const.tile([P, P], BF16, name="ones_f")
        nc.vector.memset(ones_f[:], 1.0)

        eps_t = const.tile([P, 1], F32, name="eps_t")
        nc.vector.memset(eps_t[:], EPS)

        # params -> [128, n] tiles
        def ldvec(dt_ap, n, name):
            t = const.tile([P, n], F32, tag=name, name=name)
            nc.sync.dma_start(out=t[:], in_=dt_ap.rearrange("(a p) -> p a", p=P))
            return t

        t_ln1g = [ldvec(d_ln1g.ap()[l], NE, f"ln1g{l}") for l in range(L)]
        t_ln1b = [ldvec(d_ln1b.ap()[l], NE, f"ln1b{l}") for l in range(L)]
        t_ln2g = [ldvec(d_ln2g.ap()[l], NE, f"ln2g{l}") for l in range(L)]
        t_ln2b = [ldvec(d_ln2b.ap()[l], NE, f"ln2b{l}") for l in range(L)]
        t_bo = [ldvec(d_bo.ap()[l], NE, f"bo{l}") for l in range(L)]
        t_b1 = [ldvec(d_b1.ap()[l], NFT, f"b1{l}") for l in range(L)]
        t_b2 = [ldvec(d_b2.ap()[l], NE, f"b2{l}") for l in range(L)]
        t_lnfg = ldvec(d_lnfg.ap(), NE, "lnfg")
        t_lnfb = ldvec(d_lnfb.ap(), NE, "lnfb")
        t_blm = ldvec(d_blm.ap(), NVT, "blm")

        # causal masks (bf16 multiplicative, per-core data; zigzag unit order)
        mask_t = []
        for u in range(24):
            m = pp_msk.tile([P, S // 2], BF16, tag=f"msk{u}", name=f"msk{u}")
            nc.sync.dma_start(out=m[:], in_=d_msk.ap()[u])
            mask_t.append(m)

        # residual stream xT: 8 tiles [128, S]
        xT = [pp_x.tile([P, S], F32, tag=f"x{e}", name=f"x{e}") for e in range(NE)]
        sum_t = [pp_sum.tile([P, S], F32, tag=f"s{e}", name=f"s{e}") for e in range(NE)]

        # ---- embedding gather + transpose into xT ----
        idx_t = const.tile([P, S // P], I32, name="idx_t")
        nc.sync.dma_start(out=idx_t[:], in_=d_idx.ap().rearrange("(g p) -> p g", p=P))
        xg_t = []
        for g in range(S // P):
            xg = tp.tile([P, E], F32, tag="embg", name="embg", bufs=4)
            nc.gpsimd.indirect_dma_start(
                out=xg[:], out_offset=None, in_=d_emb.ap(),
                in_offset=bass.IndirectOffsetOnAxis(ap=idx_t[:, g:g + 1], axis=0))
            xg_t.append(xg)
        for e in range(NE):
            pst = pacc.tile([P, S], F32, tag="ps_a", name="tpose", bufs=2)
            for g in range(S // P):
                nc.tensor.transpose(pst[:, g * P:(g + 1) * P],
                                    xg_t[g][:, e * P:(e + 1) * P], ident[:])
            nc.vector.tensor_copy(xT[e][:], pst[:])

        # ---- transformer blocks ----
        for l in range(L):
            with tc.tile_pool(name="qT", bufs=1) as pp_q, \
                 tc.tile_pool(name="oT", bufs=1) as pp_o, \
                 tc.tile_pool(name="attn", bufs=1) as ap_t, \
                 tc.tile_pool(name="kvsl", bufs=1) as vp, \
                 tc.tile_pool(name="pat", bufs=1, space="PSUM") as patt:

                xA = []
                for e in range(NE):
                    xa = ap_t.tile([P, S], BF16, tag=f"xa{e}", name=f"xa{e}")
                    nc.vector.tensor_copy(xa[:], xT[e][:])
                    xA.append(xa)

                # split K / V collective buffers (fp8): the V gather runs on the
                # collective cores while the score matmuls (K-only) proceed
                kin = dram.tile([ES], KV_DT, tag="kin", name="kin")
                vin = dram.tile([VS], KV_DT, tag="vin", name="vin")
                kout = dram.tile([CH * ES], KV_DT, tag="kout", name="kout")
                vout = dram.tile([CH * VS], KV_DT, tag="vout", name="vout")
                kview = kin[:].rearrange("(e s) -> e s", s=S)
                vview = vin[:].rearrange("(t f) -> t f", f=VROW)

                # k projection (W-stationary) -> kT fp8 -> kvin K region
                for m in range(NE):
                    if m % 2 == 0:
                        wk_pair = wp.tile([P, 2, NE, P], BF16, tag="wb", name="wk", bufs=3)
                        nc.sync.dma_start(out=wk_pair[:], in_=d_wkr.ap()[l, m // 2])
                    ps = pacc.tile([P, S], F32, tag="ps_a", name="acc", bufs=2)
                    for k in range(NE):
                        nc.tensor.matmul(ps[:], lhsT=wk_pair[:, m % 2, k, :], rhs=xA[k][:],
                                         start=(k == 0), stop=(k == NE - 1))
                    kl = tp.tile([P, S], KV_DT, tag="klocal", name="klocal", bufs=2)
                    nc.vector.tensor_copy(kl[:], ps[:])
                    nc.gpsimd.dma_start(out=kview[m * P:(m + 1) * P, :], in_=kl[:])

                nc.gpsimd.collective_compute(
                    "AllGather", OP.bypass, replica_groups=groups,
                    ins=[kin[:]], outs=[kout[:]])

                # v projection (x-stationary) -> fp8 -> vin
                for n in range(2):
                    w = wp.tile([P, NE * 512], BF16, tag="wbig", name="wv", bufs=2)
                    nc.sync.dma_start(
                        out=w[:].rearrange("p (a b) -> p a b", a=NE),
                        in_=d_wvf.ap()[l, n])
                    psv = [pacc.tile([P, 512], F32, tag="ps_a", name="acc", bufs=2)
                           for _ in range(4)]
                    for k in range(NE):
                        for mt in range(4):
                            nc.tensor.matmul(psv[mt][:], lhsT=xA[k][:, mt * P:(mt + 1) * P],
                                             rhs=w[:, k * 512:(k + 1) * 512], start=(k == 0),
                                             stop=(k == NE - 1))
                    for mt in range(4):
                        vv = tp.tile([P, 4, 2, 65], KV_DT, tag="vlocal", name="vlocal",
                                     bufs=2)
                        nc.vector.tensor_copy(
                            vv[:, :, :, 1:HD + 1],
                            psv[mt][:].rearrange("p (a h f) -> p a h f", a=4, h=2))
                        nc.vector.memset(vv[:, :, :, 0:1], 1.0)
                        nc.gpsimd.dma_start(
                            out=vview[mt * P:(mt + 1) * P,
                                      n * 520:(n + 1) * 520].rearrange(
                                "t (a x) -> t a x", a=4),
                            in_=vv[:].rearrange("p a h f -> p a (h f)"))

                nc.gpsimd.collective_compute(
                    "AllGather", OP.bypass, replica_groups=groups,
                    ins=[vin[:]], outs=[vout[:]])

                # q projection
                qT = []
                for m in range(NE):
                    if m % 2 == 0:
                        wq_pair = wp.tile([P, 2, NE, P], BF16, tag="wb", name="wq", bufs=3)
                        nc.sync.dma_start(out=wq_pair[:], in_=d_wqr.ap()[l, m // 2])
                    ps = pacc.tile([P, S], F32, tag="ps_a", name="acc", bufs=2)
                    for k in range(NE):
                        nc.tensor.matmul(ps[:], lhsT=wq_pair[:, m % 2, k, :], rhs=xA[k][:],
                                         start=(k == 0), stop=(k == NE - 1))
                    q = pp_q.tile([P, S], BF16, tag=f"q{m}", name=f"q{m}")
                    nc.vector.tensor_copy(q[:], ps[:])
                    qT.append(q)

                # attention, pair-major; all 16 key blocks on every core
                oT = []
                uidx = 0
                for p in range(NHP):
                    # batched K/V slab loads for this pair
                    kt = vp.tile([P, CH, S], KV_DT, tag="kt", name="kt", bufs=2)
                    vt = vp.tile([P, NSB, 130], KV_DT, tag="vt", name="vt", bufs=2)
                    for c in range(CH):
                        kc = kout[c * ES:(c + 1) * ES].rearrange("(e s) -> e s", s=S)
                        vc = vout[c * VS:(c + 1) * VS].rearrange("(t f) -> t f", f=VROW)
                        nc.sync.dma_start(out=kt[:, c, :], in_=kc[p * P:(p + 1) * P, :])
                        nc.sync.dma_start(
                            out=vt[:, 4 * c:4 * c + 4, :],
                            in_=vc[:, p * 130:(p + 1) * 130].rearrange(
                                "(mt t) f -> t mt f", t=P))
                    pvA = patt.tile([P, S], F32, tag="ps_b", name="pvA", bufs=2)
                    pvB = patt.tile([P, S], F32, tag="ps_b", name="pvB", bufs=2)
                    # zigzag: query group 0 (own blocks {2j,2j+1}) needs key
                    # blocks 0..7 only; group 1 ({14-2j,15-2j}) needs all 16.
                    # key block kb -> gather slab (owner core, slot)
                    HS = S // 2
                    u = 0
                    for qg in range(2):
                        nkb = 8 if qg == 0 else NSB
                        for kb in range(nkb):
                            if kb < 8:
                                cc, sl = kb // 2, kb % 2
                            else:
                                cc, sl = (15 - kb) // 2, 2 + (kb % 2)
                            j2 = sl
                            first, last = (kb == 0), (kb == nkb - 1)
                            qsl = qT[p][:, qg * HS:(qg + 1) * HS]
                            sAB = patt.tile([P, S], F32, tag="ps_s", name="sc", bufs=4)
                            nc.tensor.matmul(
                                sAB[:, 0:HS], lhsT=kt[0:HD, cc, j2 * P:(j2 + 1) * P],
                                rhs=qsl[0:HD, :], tile_position=(0, 0))
                            nc.tensor.matmul(
                                sAB[:, HS:S], lhsT=kt[HD:P, cc, j2 * P:(j2 + 1) * P],
                                rhs=qsl[HD:P, :], tile_position=(64, 0))
                            pe_t = ap_t.tile([P, S], BF16, tag="pt", name="pt", bufs=8)
                            nc.scalar.activation(pe_t[:], sAB[:], AF.Exp,
                                                 scale=HD ** -0.5)
                            meng = nc.gpsimd if (uidx % 4 == 3) else nc.vector
                            meng.tensor_tensor(out=pe_t[:, 0:HS], in0=pe_t[:, 0:HS],
                                               in1=mask_t[u][:], op=OP.mult)
                            meng.tensor_tensor(out=pe_t[:, HS:S], in0=pe_t[:, HS:S],
                                               in1=mask_t[u][:], op=OP.mult)
                            uidx += 1
                            u += 1
                            sbl = cc * 4 + sl
                            nc.tensor.matmul(
                                pvA[0:65, qg * HS:(qg + 1) * HS],
                                lhsT=vt[:, sbl, 0:65],
                                rhs=pe_t[:, 0:HS], start=first, stop=last,
                                skip_group_check=True)
                            nc.tensor.matmul(
                                pvB[0:65, qg * HS:(qg + 1) * HS],
                                lhsT=vt[:, sbl, 65:130],
                                rhs=pe_t[:, HS:S], start=first, stop=last,
                                skip_group_check=True)
                    # per-head normalize: folded denominator sits at row 0;
                    # recip there, partition-broadcast it (SBUF->SBUF on Pool),
                    # multiply. o row 0 becomes denom*recip = 1.0 and is
                    # cancelled by a zero row in the Wo chunk.
                    rdA = tp.tile([P, S], BF16, tag="rec", name="rdA", bufs=4)
                    rdB = tp.tile([P, S], BF16, tag="rec", name="rdB", bufs=4)
                    with nc.allow_low_precision(reason="softmax denom recip in bf16"):
                        nc.vector.reciprocal(rdA[0:1, :], pvA[0:1, :])
                        nc.vector.reciprocal(rdB[0:1, :], pvB[0:1, :])
                    nc.gpsimd.partition_broadcast(rdA[0:65, :], rdA[0:1, :])
                    nc.gpsimd.partition_broadcast(rdB[0:65, :], rdB[0:1, :])
                    oA = pp_o.tile([65, S], BF16, tag=f"o{p}a", name=f"o{p}a")
                    oB = pp_o.tile([65, S], BF16, tag=f"o{p}b", name=f"o{p}b")
                    nc.vector.tensor_tensor(out=oA[:], in0=pvA[0:65, :],
                                            in1=rdA[0:65, :], op=OP.mult)
                    nc.vector.tensor_tensor(out=oB[:], in0=pvB[0:65, :],
                                            in1=rdB[0:65, :], op=OP.mult)
                    oT.append(oA)
                    oT.append(oB)

                # output projection + bias + residual (fused on DVE)
                for m in range(NE):
                    if m % 2 == 0:
                        wo_pair = wp.tile([65, 2, 2 * NE, P], BF16, tag="wo2",
                                          bufs=2, name="wo",
                                          padded_shape=[P, 2, 2 * NE, P])
                        nc.sync.dma_start(out=wo_pair[:], in_=d_wor.ap()[l, m // 2])
                    ps = pacc.tile([P, S], F32, tag="ps_a", name="acc", bufs=2)
                    for k in range(2 * NE):
                        nc.tensor.matmul(ps[:], lhsT=wo_pair[:, m % 2, k, :], rhs=oT[k][:],
                                         start=(k == 0), stop=(k == 2 * NE - 1))
                    nc.vector.scalar_tensor_tensor(
                        out=sum_t[m][:], in0=ps[:], scalar=t_bo[l][:, m:m + 1],
                        in1=xT[m][:], op0=OP.add, op1=OP.add)

            _ln_tiles(nc, tc, pools, sum_t, xT, t_ln1g[l], t_ln1b[l], ones_f, eps_t)

            # FFN (bf16 matmuls, fp32 psum + residual)
            with tc.tile_pool(name="ht", bufs=1) as pp_h, \
                 tc.tile_pool(name="xbf", bufs=1) as pp_xbf:
                xF = []
                for e in range(NE):
                    xf = pp_xbf.tile([P, S], BF16, tag=f"xf{e}", name=f"xf{e}")
                    nc.vector.tensor_copy(xf[:], xT[e][:])
                    xF.append(xf)
                hT = []
                for f in range(NFT):
                    if f % 2 == 0:
                        w1_pair = wp.tile([P, 2, NE, P], BF16, tag="wb", name="w1", bufs=3)
                        nc.sync.dma_start(out=w1_pair[:], in_=d_w1r.ap()[l, f // 2])
                    ps = pacc.tile([P, S], F32, tag="ps_a", name="acc", bufs=2)
                    for k in range(NE):
                        nc.tensor.matmul(ps[:], lhsT=w1_pair[:, f % 2, k, :], rhs=xF[k][:],
                                         start=(k == 0), stop=(k == NE - 1))
                    h = pp_h.tile([P, S], BF16, tag=f"h{f}", name=f"h{f}")
                    nc.scalar.activation(h[:], ps[:], AF.Relu, bias=t_b1[l][:, f:f + 1])
                    hT.append(h)
                for m in range(NE):
                    w = wp.tile([P, NFT * P], BF16, tag="wbig", name="w2", bufs=2)
                    nc.sync.dma_start(
                        out=w[:].rearrange("p (a b) -> p a b", a=NFT),
                        in_=d_w2r.ap()[l, m])
                    ps = pacc.tile([P, S], F32, tag="ps_a", name="acc", bufs=2)
                    for f in range(NFT):
                        nc.tensor.matmul(ps[:], lhsT=w[:, f * P:(f + 1) * P], rhs=hT[f][:],
                                         start=(f == 0), stop=(f == NFT - 1))
                    nc.vector.scalar_tensor_tensor(
                        out=sum_t[m][:], in0=ps[:], scalar=t_b2[l][:, m:m + 1],
                        in1=xT[m][:], op0=OP.add, op1=OP.add)
            _ln_tiles(nc, tc, pools, sum_t, xT, t_ln2g[l], t_ln2b[l], ones_f, eps_t)

        # ---- final LN + lm_head ----
        _ln_tiles(nc, tc, pools, xT, sum_t, t_lnfg, t_lnfb, ones_f, eps_t)
        with tc.tile_pool(name="lg", bufs=1) as pp_lg, \
             tc.tile_pool(name="xb", bufs=1) as pp_xb, \
             tc.tile_pool(name="plm", bufs=1, space="PSUM") as plm:
            xB = []
            for e in range(NE):
                xb = pp_xb.tile([P, S], BF16, tag=f"xb{e}", name=f"xb{e}")
                nc.vector.tensor_copy(xb[:], sum_t[e][:])
                xB.append(xb)
            for vt2 in range(0, NVT, 2):
                w = wp.tile([P, 2, NE, P], BF16, tag="wb", name="wlmt", bufs=3)
                nc.sync.dma_start(out=w[:], in_=d_wlm.ap()[vt2 // 2])
                ps = plm.tile([P, 2 * S], F32, tag="ps_lm", name="acc", bufs=3)
                for g in range(2):
                    for k in range(NE):
                        nc.tensor.matmul(ps[:, g * S:(g + 1) * S], lhsT=w[:, g, k, :],
                                         rhs=xB[k][:], start=(k == 0),
                                         stop=(k == NE - 1), skip_group_check=True)
                lg = pp_lg.tile([P, 2, S], F32, tag="lg", name="lg", bufs=4)
                nc.scalar.add(lg[:, 0, :], ps[:, 0:S], t_blm[:, vt2:vt2 + 1])
                nc.vector.tensor_scalar(lg[:, 1, :], ps[:, S:2 * S],
                                        t_blm[:, vt2 + 1:vt2 + 2], None, OP.add)
                nc.scalar.dma_start(
                    out=d_out.ap()[vt2 * P:(vt2 + 2) * P, :].rearrange(
                        "(g p) s -> p g s", p=P),
                    in_=lg[:])

    nc.compile()
    return nc


_CACHED = {}


def _swz(w_me):
    """[M_out_tiles, E_in, P] -> [M/2, P(part), 2, NE, P]: the exact SBUF tile
    layout for the W-stationary projection loops, so DMAs are contiguous."""
    m, e, p = w_me.shape
    # element [m2, part, g, ko, f] = w_me[m2*2+g, ko*128+part, f]
    return np.ascontiguousarray(
        w_me.reshape(m // 2, 2, NE, P, p).transpose(0, 3, 1, 2, 4))


def _prep_weights(inputs):
    f32 = np.float32
    bf = ml_dtypes.bfloat16
    Wq, Wk, Wv = inputs["Wq"], inputs["Wk"], inputs["Wv"]
    wq_flat = np.ascontiguousarray(Wq.transpose(0, 2, 1, 3).reshape(L, E, H * HD))
    wk_flat = np.ascontiguousarray(Wk.transpose(0, 2, 1, 3).reshape(L, E, H * HD))
    wv_flat = np.ascontiguousarray(Wv.transpose(0, 2, 1, 3).reshape(L, E, H * HD))
    wqr = np.stack([_swz(wq_flat[l].reshape(E, NE, P).transpose(1, 0, 2))
                    for l in range(L)]).astype(f32)
    wkr = np.stack([_swz(wk_flat[l].reshape(E, NE, P).transpose(1, 0, 2))
                    for l in range(L)]).astype(f32)
    # Wo in 65-row head chunks (row 0 zero): [l, m2, d, g, c, f] =
    #   0 if d==0 else Wo[l, c*64+(d-1), (2*m2+g)*128+f]
    wo_c = inputs["Wo"].reshape(L, 16, HD, E)
    wo_z = np.concatenate([np.zeros((L, 16, 1, E), wo_c.dtype), wo_c], axis=2)
    wor = np.ascontiguousarray(
        wo_z.reshape(L, 16, 65, NE // 2, 2, P).transpose(
            0, 3, 2, 4, 1, 5)).astype(f32)
    w1r = np.stack([_swz(inputs["W1"][l].reshape(E, NFT, P).transpose(1, 0, 2))
                    for l in range(L)]).astype(f32)
    # wvf: [L, 2, P(part), NE(ko), 512] -- element = Wv_flat[l, ko*128+part, n*512+f]
    wvf = np.ascontiguousarray(
        wv_flat.reshape(L, NE, P, 2, 512).transpose(0, 3, 2, 1, 4)).astype(f32)
    # w2r: [L, NE(m), P(part), NFT, P] -- element = W2[l, ff*128+part, m*128+col]
    w2r = np.ascontiguousarray(
        inputs["W2"].reshape(L, NFT, P, NE, P).transpose(0, 3, 2, 1, 4)).astype(f32)
    wlm_me = np.ascontiguousarray(
        inputs["Wlm"].reshape(E, NVT, P).transpose(1, 0, 2))
    wlmr = _swz(wlm_me).astype(f32)
    return {
        "wqr": wqr.astype(bf), "wkr": wkr.astype(bf), "wvf": wvf.astype(bf),
        "wor": wor.astype(bf), "w1r": w1r.astype(bf), "w2r": w2r.astype(bf),
        "wlmr": wlmr.astype(bf),
        "emb": np.ascontiguousarray(inputs["emb"]).astype(f32),
        "ln1g": np.ascontiguousarray(inputs["ln1_g"]).astype(f32),
        "ln1b": np.ascontiguousarray(inputs["ln1_b"]).astype(f32),
        "ln2g": np.ascontiguousarray(inputs["ln2_g"]).astype(f32),
        "ln2b": np.ascontiguousarray(inputs["ln2_b"]).astype(f32),
        "bo": np.ascontiguousarray(inputs["bo"]).astype(f32),
        "b1": np.ascontiguousarray(inputs["b1"]).astype(f32),
        "b2": np.ascontiguousarray(inputs["b2"]).astype(f32),
        "lnfg": np.ascontiguousarray(inputs["lnf_g"]).astype(f32),
        "lnfb": np.ascontiguousarray(inputs["lnf_b"]).astype(f32),
        "blm": np.ascontiguousarray(inputs["blm"]).astype(f32),
    }


def _zig_blocks(j):
    return [2 * j, 2 * j + 1, 14 - 2 * j, 15 - 2 * j]


def kernel(**inputs):
    if "nc" not in _CACHED:
        _CACHED["nc"] = build_program()
    nc = _CACHED["nc"]

    shared = _prep_weights(inputs)
    index = np.asarray(inputs["index"])

    # per-core zigzag token ids and causal masks in attention-unit order
    ipos = np.arange(P)
    in_maps = []
    for c in range(NC):
        b, j = c // CH, c % CH
        blocks = _zig_blocks(j)
        tok = np.concatenate([np.arange(bk * P, (bk + 1) * P) for bk in blocks])
        qpos = np.concatenate([tok[0:256], tok[256:512]])
        m = np.zeros((24, P, S // 2), np.float32)
        u = 0
        for qg in range(2):
            nkb = 8 if qg == 0 else NSB
            qp = qpos[qg * 256:(qg + 1) * 256]
            for kb in range(nkb):
                m[u] = ((kb * P + ipos)[:, None] <= qp[None, :])
                u += 1
        im = dict(shared)
        im["maskp"] = m.astype(ml_dtypes.bfloat16)
        im["idx"] = np.ascontiguousarray(index[b, tok]).astype(np.int32)
        in_maps.append(im)

    res = bass_utils.run_bass_kernel_spmd(nc, in_maps, core_ids=list(range(NC)))
    out = np.zeros((B, T, V), np.float32)
    for c in range(NC):
        b, j = c // CH, c % CH
        blocks = _zig_blocks(j)
        tok = np.concatenate([np.arange(bk * P, (bk + 1) * P) for bk in blocks])
        out[b, tok, :] = res.results[c]["logt"].T
    return out
